# revision 26
# baseline (speedup 1.0000x reference)
"""Causal self-attention on 8 trn2 NeuronCores.

Sharding: tensor-parallel over heads (2 heads per core, both batches).
Each core computes Q/K/V projections for its heads (column-parallel),
causal attention, and a row-parallel partial of the output projection.
Host sums the 8 partials and adds the bias terms (bo, and bv@Wo which
is exact because softmax rows sum to 1).

v2 changes vs the 439us baseline:
  - all matmuls bf16 (same PE rate as fp32r, FWL weight loads, half DMA)
  - consolidated DMAs: one or two descriptors per weight tensor, one
    16KB/partition descriptor per (batch, chunk) of x; host pre-packs
    layouts so no small-element gather DMAs remain
  - dedicated DMA queues: sync=x chunks, gpsimd=weights+output
  - PE warmup matmul stream at t=0 so HAM reaches K=8/8 before real work
  - paired score tiles: one [128,1024] PSUM (2 banks) per two key tiles,
    one exp instruction covering both (halves ACT per-instruction cost)
  - softmax denominator: exp tiles accumulated on DVE into E, a single
    ones-matmul per (head, chunk) instead of one per key tile
  - output staged [128,2048] bf16 in SBUF, one DMA per row block

Layout choices (partition dim first):
  xt   : x transposed -> (model 128-blocks on partitions, seq free)
  Qt/Kt: (head_dim on partitions, seq free)        [proj lhsT = W block]
  V    : (seq keys on partitions, head_dim free)   [proj lhsT = xt block]
  scores: St = (keys, queries) tiles = Kt_blk.T @ Qt_chunk
  exp(St) feeds P@V directly:  attnT = V_blk.T @ exp  (head_dim, queries)
  denominator: ones(128,128).T @ E where E = sum_i exp_i (DVE)
  out-proj: lhsT = attnT block, rhs = Wo rows slice -> (queries, model)
"""

import os

import numpy as np

import concourse.bass as bass
import concourse.mybir as mybir
import concourse.tile as tile
from concourse import bacc
from concourse.bass_utils import run_bass_kernel_spmd

F32 = mybir.dt.float32
BF16 = mybir.dt.bfloat16
AF = mybir.ActivationFunctionType
ALU = mybir.AluOpType

B = 2
S = 2048
D = 2048
H = 16
DH = 128
NCORES = 8
HPC = H // NCORES  # heads per core = 2
KT = D // 128  # 16 contraction tiles for projections
NQC = S // 512  # 4 query chunks per sequence
SCALE = 1.0 / np.sqrt(DH)
NEG = -1e9

_NC_CACHE = {}


def _build():
    nc = bacc.Bacc(None, target_bir_lowering=False, debug=False)

    # host pre-packed inputs (see kernel() below)
    xt = nc.dram_tensor("xt", [B, NQC, 128, KT * 512], BF16,
                        kind="ExternalInput")
    wq = nc.dram_tensor("wq", [128, KT * HPC * DH], BF16,
                        kind="ExternalInput")
    wk = nc.dram_tensor("wk", [128, KT * HPC * DH], BF16,
                        kind="ExternalInput")
    wv = nc.dram_tensor("wv", [128, KT * HPC * DH], BF16,
                        kind="ExternalInput")
    wo = nc.dram_tensor("wo", [128, HPC * D], BF16, kind="ExternalInput")
    bq2 = nc.dram_tensor("bq2", [128, HPC], F32, kind="ExternalInput")
    bk2 = nc.dram_tensor("bk2", [128, HPC], F32, kind="ExternalInput")
    mblk = nc.dram_tensor("mblk", [128, 128], F32, kind="ExternalInput")
    onem = nc.dram_tensor("onem", [128, 128], BF16, kind="ExternalInput")
    out = nc.dram_tensor("out", [B, S, D], BF16, kind="ExternalOutput")

    with tile.TileContext(nc) as tc:
        with (
            tc.tile_pool(name="const", bufs=1) as constp,
            tc.tile_pool(name="xtp", bufs=3) as xtp,
            tc.tile_pool(name="qkv", bufs=2) as qkvp,
            tc.tile_pool(name="expp", bufs=6) as expp,
            tc.tile_pool(name="ep", bufs=2) as ep,
            tc.tile_pool(name="attnp", bufs=5) as attnp,
            tc.tile_pool(name="ostp", bufs=3) as ostp,
            # PSUM bank budget (8): st-pairs 2x2 + attn 2 + misc 2
            tc.tile_pool(name="ps_st", bufs=2, space="PSUM") as ps_st,
            tc.tile_pool(name="ps_at", bufs=2, space="PSUM") as ps_at,
            tc.tile_pool(name="ps_ms", bufs=2, space="PSUM") as ps_ms,
        ):
            # ---- constants: critical first-chunk weights first ----
            bq_t = constp.tile([128, HPC], F32, tag="bq")
            bk_t = constp.tile([128, HPC], F32, tag="bk")
            mask_t = constp.tile([128, 128], F32, tag="mask")
            ones_m = constp.tile([128, 128], BF16, tag="ones_m")

            # wq/wk packed per-head: [128, h*(KT*DH) + k*DH + d] so the
            # first half-DMA delivers all of head 0 (consumption order).
            HKD = KT * DH  # 2048 = one head's weight cols
            HD = HPC * DH
            wq_t = constp.tile([128, HPC * HKD], BF16, tag="wq")
            wk_t = constp.tile([128, HPC * HKD], BF16, tag="wk")
            wv_t = constp.tile([128, KT * HD], BF16, tag="wv")
            wo_t = constp.tile([128, HPC * D], BF16, tag="wo")
            nc.gpsimd.dma_start(wq_t[:, :HKD], wq[:, :HKD])
            nc.gpsimd.dma_start(wk_t[:, :HKD], wk[:, :HKD])
            nc.gpsimd.dma_start(bq_t[:], bq2[:])
            nc.gpsimd.dma_start(bk_t[:], bk2[:])
            nc.gpsimd.dma_start(mask_t[:], mblk[:])
            nc.gpsimd.dma_start(ones_m[:], onem[:])
            nc.gpsimd.dma_start(wq_t[:, HKD:], wq[:, HKD:])
            nc.gpsimd.dma_start(wk_t[:, HKD:], wk[:, HKD:])
            hw = KT * HD // 2
            nc.gpsimd.dma_start(wv_t[:, :hw], wv[:, :hw])
            nc.gpsimd.dma_start(wv_t[:, hw:], wv[:, hw:])
            nc.gpsimd.dma_start(
                wo_t[:, : HPC * D // 2], wo[:, : HPC * D // 2]
            )
            nc.gpsimd.dma_start(
                wo_t[:, HPC * D // 2 :], wo[:, HPC * D // 2 :]
            )

            # ---- warmup: ACT exp table + ~5us of PE activity ----
            warm_sb = constp.tile([128, 512], BF16, tag="warm_sb")
            nc.vector.memset(warm_sb[:], 0)
            warm_t = constp.tile([128, 1], F32, tag="warm")
            nc.scalar.activation(warm_t[:], warm_sb[:, 0:1], AF.Exp,
                                 scale=0.0)
            warm_ps = ps_ms.tile([128, 512], F32, tag="ms", name="warm_ps")
            NWARM = 20
            for i in range(NWARM):
                nc.tensor.matmul(
                    warm_ps[:],
                    warm_sb[:, :128],
                    warm_sb[:],
                    start=(i == 0),
                    stop=(i == NWARM - 1),
                )

            # ---- chunk sequence ----
            chunks = [(b, c) for b in range(B) for c in range(NQC)]
            xtiles = {}

            def emit_xt_dma(idx):
                if idx >= len(chunks):
                    return
                b, c = chunks[idx]
                x_t = xtp.tile([128, KT * 512], BF16, tag="xt",
                               name=f"xt{b}_{c}")
                qtr = KT * 512 // 4
                for q in range(4):
                    nc.sync.dma_start(
                        x_t[:, q * qtr : (q + 1) * qtr],
                        xt[b, c, :, q * qtr : (q + 1) * qtr],
                    )
                xtiles[idx] = x_t

            emit_xt_dma(0)
            emit_xt_dma(1)

            # per-batch persistent tiles (double-buffered across batches)
            qkts = {}

            def get_qkv(b):
                if b not in qkts:
                    qt = qkvp.tile([128, HPC, S], BF16, tag="qt",
                                   name=f"qt{b}")
                    ktt = qkvp.tile([128, HPC, S], BF16, tag="ktt",
                                    name=f"ktt{b}")
                    vt = qkvp.tile([128, S // 128, HPC * DH], BF16, tag="vt",
                                   name=f"vt{b}")
                    qkts[b] = (qt, ktt, vt)
                return qkts[b]

            def p_chunk(idx):
                b, c = chunks[idx]
                emit_xt_dma(idx + 2)
                x_t = xtiles.pop(idx)
                qt, ktt, vt = get_qkv(b)
                c0 = c * 512
                HD = HPC * DH
                HKD = KT * DH
                for h in range(HPC):
                    for w_t, dst, bias_t in (
                        (wq_t, qt, bq_t),
                        (wk_t, ktt, bk_t),
                    ):
                        ps_t = ps_st.tile([128, 1024], F32, tag="st",
                                          name="qk_ps")
                        ps = ps_t[:, :512]
                        for k in range(KT):
                            nc.tensor.matmul(
                                ps,
                                w_t[:, h * HKD + k * DH : h * HKD
                                    + (k + 1) * DH],
                                x_t[:, k * 512 : (k + 1) * 512],
                                start=(k == 0),
                                stop=(k == KT - 1),
                            )
                        nc.vector.tensor_scalar_add(
                            dst[:, h, c0 : c0 + 512],
                            ps,
                            bias_t[:, h : h + 1],
                        )
                for s in range(4):
                    ps = ps_ms.tile([128, HD], F32, tag="ms",
                                    name="v_ps")
                    for k in range(KT):
                        nc.tensor.matmul(
                            ps[:],
                            x_t[:, k * 512 + s * 128 : k * 512
                                + (s + 1) * 128],
                            wv_t[:, k * HD : (k + 1) * HD],
                            start=(k == 0),
                            stop=(k == KT - 1),
                        )
                    nc.scalar.copy(vt[:, c * 4 + s, :], ps[:])

            def a_chunk(idx):
                b, qc = chunks[idx]
                qt, ktt, vt = get_qkv(b)
                n_kt = 4 * (qc + 1)
                at_sb = []
                for h in range(HPC):
                    hsl = slice(h * DH, (h + 1) * DH)
                    attn_ps = ps_at.tile([128, 512], F32, tag="at",
                                         name="attn_ps")
                    e_sb = ep.tile([128, 512], BF16, tag="E", name="e_sb")
                    for p in range(n_kt // 2):
                        i0, i1 = 2 * p, 2 * p + 1
                        lo0 = 128 * (i0 - 4 * qc) if i0 >= 4 * qc else 0
                        lo1 = 128 * (i1 - 4 * qc) if i1 >= 4 * qc else 0
                        st = ps_st.tile([128, 1024], F32, tag="st",
                                        name="st")
                        ex = expp.tile([128, 1024], BF16, tag="exp",
                                       name="ex")
                        for j, (i, lo) in enumerate(((i0, lo0), (i1, lo1))):
                            off = j * 512
                            nc.tensor.matmul(
                                st[:, off + lo : off + 512],
                                ktt[:, h, i * 128 : (i + 1) * 128],
                                qt[:, h, qc * 512 + lo : (qc + 1) * 512],
                                start=True,
                                stop=True,
                            )
                            if i >= 4 * qc:  # diagonal tile: add mask
                                nc.vector.tensor_tensor(
                                    st[:, off + lo : off + lo + 128],
                                    st[:, off + lo : off + lo + 128],
                                    mask_t[:],
                                    op=ALU.add,
                                )
                        # one exp over both halves (middle cols unused)
                        nc.scalar.activation(
                            ex[:, lo0:], st[:, lo0:], AF.Exp, scale=SCALE
                        )
                        # accumulate denominator tile E on DVE (bf16 2x)
                        if p == 0:
                            nc.vector.tensor_copy(
                                e_sb[:, lo0:], ex[:, lo0:512]
                            )
                        else:
                            nc.vector.tensor_tensor(
                                e_sb[:, lo0:],
                                e_sb[:, lo0:],
                                ex[:, lo0:512],
                                op=ALU.add,
                            )
                        nc.vector.tensor_tensor(
                            e_sb[:, lo1:],
                            e_sb[:, lo1:],
                            ex[:, 512 + lo1 :],
                            op=ALU.add,
                        )
                        nc.tensor.matmul(
                            attn_ps[:, lo0:],
                            vt[:, i0, hsl],
                            ex[:, lo0:512],
                            start=(p == 0),
                            stop=False,
                        )
                        nc.tensor.matmul(
                            attn_ps[:, lo1:],
                            vt[:, i1, hsl],
                            ex[:, 512 + lo1 :],
                            start=False,
                            stop=(p == n_kt // 2 - 1),
                        )
                    # rbden lives in the st pool (a half pair-tile):
                    # keeps the ms ring free for proj/outproj groups
                    rbden_t = ps_st.tile([128, 1024], F32, tag="st",
                                         name="rbden")
                    rbden = rbden_t[:, :512]
                    nc.tensor.matmul(
                        rbden, ones_m[:], e_sb[:], start=True, stop=True
                    )
                    rc_sb = attnp.tile([128, 512], F32, tag="rc",
                                       name="rc_sb")
                    nc.vector.reciprocal_approx_fast(
                        out=rc_sb[:], in_=rbden
                    )
                    a_sb = attnp.tile([128, 512], BF16, tag="attnT",
                                      name="a_sb")
                    nc.vector.tensor_tensor(
                        a_sb[:], attn_ps[:], rc_sb[:], op=ALU.mult
                    )
                    at_sb.append(a_sb)
                return at_sb

            def outproj(idx, at_sb):
                b, qc = chunks[idx]
                for qs in range(4):
                    row0 = qc * 512 + qs * 128
                    o_t = ostp.tile([128, D], BF16, tag="ost", name="o_t")
                    for nch in range(4):
                        ps = ps_ms.tile([128, 512], F32, tag="ms",
                                        name="op_ps")
                        for h in range(HPC):
                            nc.tensor.matmul(
                                ps[:],
                                at_sb[h][:, qs * 128 : (qs + 1) * 128],
                                wo_t[:, h * D + nch * 512 : h * D
                                     + (nch + 1) * 512],
                                start=(h == 0),
                                stop=(h == HPC - 1),
                            )
                        osl = o_t[:, nch * 512 : (nch + 1) * 512]
                        if nch % 2 == 0:
                            nc.vector.tensor_copy(osl, ps[:])
                        else:
                            nc.scalar.copy(osl, ps[:])
                    nc.gpsimd.dma_start(
                        out[b, row0 : row0 + 128, :], o_t[:]
                    )

            pending = None
            for idx in range(len(chunks)):
                p_chunk(idx)
                if pending is not None:
                    outproj(pending[0], pending[1])
                at = a_chunk(idx)
                pending = (idx, at)
            outproj(pending[0], pending[1])
    nc.compile()
    return nc


def _get_nc():
    if "nc" not in _NC_CACHE:
        _NC_CACHE["nc"] = _build()
    return _NC_CACHE["nc"]


def kernel(x, mask, Wq, bq, Wk, bk, Wv, bv, Wo, bo):
    import ml_dtypes

    bf16 = ml_dtypes.bfloat16
    x = np.asarray(x, dtype=np.float32)
    Wq = np.asarray(Wq, dtype=np.float32)
    Wk = np.asarray(Wk, dtype=np.float32)
    Wv = np.asarray(Wv, dtype=np.float32)
    Wo = np.asarray(Wo, dtype=np.float32)
    bq = np.asarray(bq, dtype=np.float32)
    bk = np.asarray(bk, dtype=np.float32)
    bv = np.asarray(bv, dtype=np.float32)
    bo = np.asarray(bo, dtype=np.float32)

    nc = _get_nc()

    # x -> [B, NQC, 128(model block), KT*512] with model on partitions:
    # element (b, c, p, k*512+s) = x[b, c*512+s, k*128+p]
    xt_np = (
        np.ascontiguousarray(
            x.reshape(B, NQC, 512, KT, 128).transpose(0, 1, 4, 3, 2)
        )
        .reshape(B, NQC, 128, KT * 512)
        .astype(bf16)
    )
    kl = np.arange(128)
    mblk = np.where(kl[:, None] <= kl[None, :], 0.0, NEG).astype(np.float32)
    onem = np.ones((128, 128), dtype=bf16)

    def pack_w(W, cols):
        # k-major: [128, k*(HPC*DH) + hd]; partition p holds row k*128+p
        return np.ascontiguousarray(
            W[:, cols].reshape(KT, 128, HPC * DH).transpose(1, 0, 2)
        ).reshape(128, KT * HPC * DH).astype(bf16)

    def pack_w_headmajor(W, cols):
        # head-major: [128, h*(KT*DH) + k*DH + d]
        w = W[:, cols].reshape(KT, 128, HPC, DH)  # (k, p, h, d)
        return np.ascontiguousarray(
            w.transpose(1, 2, 0, 3)  # (p, h, k, d)
        ).reshape(128, HPC * KT * DH).astype(bf16)

    in_maps = []
    for c in range(NCORES):
        cols = slice(c * HPC * DH, (c + 1) * HPC * DH)
        wo_c = np.ascontiguousarray(
            Wo[cols, :].reshape(HPC, 128, D).transpose(1, 0, 2)
        ).reshape(128, HPC * D).astype(bf16)
        in_maps.append(
            {
                "xt": xt_np,
                "wq": pack_w_headmajor(Wq, cols),
                "wk": pack_w_headmajor(Wk, cols),
                "wv": pack_w(Wv, cols),
                "wo": wo_c,
                "bq2": np.ascontiguousarray(
                    bq[cols].reshape(HPC, 128).T
                ),
                "bk2": np.ascontiguousarray(
                    bk[cols].reshape(HPC, 128).T
                ),
                "mblk": mblk,
                "onem": onem,
            }
        )

    trace = bool(int(os.environ.get("BASS_ATTN_TRACE", "0")))
    try:
        res = run_bass_kernel_spmd(
            nc, in_maps, core_ids=list(range(NCORES)), trace=trace
        )
    except Exception:
        # transient device errors (e.g. a wedged core from a prior run)
        # usually clear on retry
        res = run_bass_kernel_spmd(
            nc, in_maps, core_ids=list(range(NCORES)), trace=trace
        )
    if trace:
        _NC_CACHE["last_result"] = res

    acc = res.results[0]["out"].astype(np.float64)
    for c in range(1, NCORES):
        acc += res.results[c]["out"].astype(np.float64)
    # bv's effect: softmax rows sum to 1, so attn = attn_nobv + bv per head
    # -> out += bv @ Wo (exact). bo added directly.
    corr = (bv.astype(np.float64) @ Wo.astype(np.float64)) + bo.astype(
        np.float64
    )
    acc += corr
    return acc.astype(np.float32)


# revision 28
# speedup vs baseline: 1.0242x; 1.0242x over previous
"""Causal self-attention on 8 trn2 NeuronCores.

Sharding: tensor-parallel over heads (2 heads per core, both batches).
Each core computes Q/K/V projections for its heads (column-parallel),
causal attention, and a row-parallel partial of the output projection.
Host sums the 8 partials and adds the bias terms (bo, and bv@Wo which
is exact because softmax rows sum to 1).

v2 changes vs the 439us baseline:
  - all matmuls bf16 (same PE rate as fp32r, FWL weight loads, half DMA)
  - consolidated DMAs: one or two descriptors per weight tensor, one
    16KB/partition descriptor per (batch, chunk) of x; host pre-packs
    layouts so no small-element gather DMAs remain
  - dedicated DMA queues: sync=x chunks, gpsimd=weights+output
  - PE warmup matmul stream at t=0 so HAM reaches K=8/8 before real work
  - paired score tiles: one [128,1024] PSUM (2 banks) per two key tiles,
    one exp instruction covering both (halves ACT per-instruction cost)
  - softmax denominator: exp tiles accumulated on DVE into E, a single
    ones-matmul per (head, chunk) instead of one per key tile
  - output staged [128,2048] bf16 in SBUF, one DMA per row block

Layout choices (partition dim first):
  xt   : x transposed -> (model 128-blocks on partitions, seq free)
  Qt/Kt: (head_dim on partitions, seq free)        [proj lhsT = W block]
  V    : (seq keys on partitions, head_dim free)   [proj lhsT = xt block]
  scores: St = (keys, queries) tiles = Kt_blk.T @ Qt_chunk
  exp(St) feeds P@V directly:  attnT = V_blk.T @ exp  (head_dim, queries)
  denominator: ones(128,128).T @ E where E = sum_i exp_i (DVE)
  out-proj: lhsT = attnT block, rhs = Wo rows slice -> (queries, model)
"""

import os

import numpy as np

import concourse.bass as bass
import concourse.mybir as mybir
import concourse.tile as tile
from concourse import bacc
from concourse.bass_utils import run_bass_kernel_spmd

F32 = mybir.dt.float32
BF16 = mybir.dt.bfloat16
AF = mybir.ActivationFunctionType
ALU = mybir.AluOpType

B = 2
S = 2048
D = 2048
H = 16
DH = 128
NCORES = 8
HPC = H // NCORES  # heads per core = 2
KT = D // 128  # 16 contraction tiles for projections
NQC = S // 512  # 4 query chunks per sequence
SCALE = 1.0 / np.sqrt(DH)
NEG = -1e9

_NC_CACHE = {}


def _build():
    nc = bacc.Bacc(None, target_bir_lowering=False, debug=False)

    # host pre-packed inputs (see kernel() below)
    xt = nc.dram_tensor("xt", [B, NQC, 128, KT * 512], BF16,
                        kind="ExternalInput")
    wq = nc.dram_tensor("wq", [128, KT * HPC * DH], BF16,
                        kind="ExternalInput")
    wk = nc.dram_tensor("wk", [128, KT * HPC * DH], BF16,
                        kind="ExternalInput")
    wv = nc.dram_tensor("wv", [128, KT * HPC * DH], BF16,
                        kind="ExternalInput")
    wo = nc.dram_tensor("wo", [128, HPC * D], BF16, kind="ExternalInput")
    bq2 = nc.dram_tensor("bq2", [128, HPC], F32, kind="ExternalInput")
    bk2 = nc.dram_tensor("bk2", [128, HPC], F32, kind="ExternalInput")
    mblk = nc.dram_tensor("mblk", [128, 128], F32, kind="ExternalInput")
    onem = nc.dram_tensor("onem", [128, 128], BF16, kind="ExternalInput")
    out = nc.dram_tensor("out", [B, S, D], BF16, kind="ExternalOutput")

    with tile.TileContext(nc) as tc:
        with (
            tc.tile_pool(name="const", bufs=1) as constp,
            tc.tile_pool(name="xtp", bufs=3) as xtp,
            tc.tile_pool(name="qkv", bufs=2) as qkvp,
            tc.tile_pool(name="expp", bufs=6) as expp,
            tc.tile_pool(name="ep", bufs=2) as ep,
            tc.tile_pool(name="attnp", bufs=5) as attnp,
            tc.tile_pool(name="ostp", bufs=4) as ostp,
            # PSUM bank budget (8): st-pairs 2x2 + attn 2 + misc 2
            tc.tile_pool(name="ps_st", bufs=2, space="PSUM") as ps_st,
            tc.tile_pool(name="ps_at", bufs=2, space="PSUM") as ps_at,
            tc.tile_pool(name="ps_ms", bufs=2, space="PSUM") as ps_ms,
        ):
            # ---- constants: critical first-chunk weights first ----
            bq_t = constp.tile([128, HPC], F32, tag="bq")
            bk_t = constp.tile([128, HPC], F32, tag="bk")
            mask_t = constp.tile([128, 128], F32, tag="mask")
            ones_m = constp.tile([128, 128], BF16, tag="ones_m")

            # wq/wk packed per-head: [128, h*(KT*DH) + k*DH + d] so the
            # first half-DMA delivers all of head 0 (consumption order).
            HKD = KT * DH  # 2048 = one head's weight cols
            HD = HPC * DH
            wq_t = constp.tile([128, HPC * HKD], BF16, tag="wq")
            wk_t = constp.tile([128, HPC * HKD], BF16, tag="wk")
            wv_t = constp.tile([128, KT * HD], BF16, tag="wv")
            wo_t = constp.tile([128, HPC * D], BF16, tag="wo")
            nc.gpsimd.dma_start(wq_t[:, :HKD], wq[:, :HKD])
            nc.gpsimd.dma_start(wk_t[:, :HKD], wk[:, :HKD])
            nc.gpsimd.dma_start(bq_t[:], bq2[:])
            nc.gpsimd.dma_start(bk_t[:], bk2[:])
            nc.gpsimd.dma_start(mask_t[:], mblk[:])
            nc.gpsimd.dma_start(ones_m[:], onem[:])
            nc.gpsimd.dma_start(wq_t[:, HKD:], wq[:, HKD:])
            nc.gpsimd.dma_start(wk_t[:, HKD:], wk[:, HKD:])
            hw = KT * HD // 2
            nc.gpsimd.dma_start(wv_t[:, :hw], wv[:, :hw])
            nc.gpsimd.dma_start(wv_t[:, hw:], wv[:, hw:])
            nc.gpsimd.dma_start(
                wo_t[:, : HPC * D // 2], wo[:, : HPC * D // 2]
            )
            nc.gpsimd.dma_start(
                wo_t[:, HPC * D // 2 :], wo[:, HPC * D // 2 :]
            )

            # ---- warmup: ACT exp table + ~5us of PE activity ----
            warm_sb = constp.tile([128, 512], BF16, tag="warm_sb")
            nc.vector.memset(warm_sb[:], 0)
            warm_t = constp.tile([128, 1], F32, tag="warm")
            nc.scalar.activation(warm_t[:], warm_sb[:, 0:1], AF.Exp,
                                 scale=0.0)
            warm_ps = ps_ms.tile([128, 512], F32, tag="ms", name="warm_ps")
            NWARM = 14
            for i in range(NWARM):
                nc.tensor.matmul(
                    warm_ps[:],
                    warm_sb[:, :128],
                    warm_sb[:],
                    start=(i == 0),
                    stop=(i == NWARM - 1),
                )

            # ---- chunk sequence ----
            chunks = [(b, c) for b in range(B) for c in range(NQC)]
            xtiles = {}

            def emit_xt_dma(idx):
                if idx >= len(chunks):
                    return
                b, c = chunks[idx]
                x_t = xtp.tile([128, KT * 512], BF16, tag="xt",
                               name=f"xt{b}_{c}")
                qtr = KT * 512 // 4
                for q in range(4):
                    nc.sync.dma_start(
                        x_t[:, q * qtr : (q + 1) * qtr],
                        xt[b, c, :, q * qtr : (q + 1) * qtr],
                    )
                xtiles[idx] = x_t

            emit_xt_dma(0)
            emit_xt_dma(1)

            # per-batch persistent tiles (double-buffered across batches)
            qkts = {}

            def get_qkv(b):
                if b not in qkts:
                    qt = qkvp.tile([128, HPC, S], BF16, tag="qt",
                                   name=f"qt{b}")
                    ktt = qkvp.tile([128, HPC, S], BF16, tag="ktt",
                                    name=f"ktt{b}")
                    vt = qkvp.tile([128, S // 128, HPC * DH], BF16, tag="vt",
                                   name=f"vt{b}")
                    qkts[b] = (qt, ktt, vt)
                return qkts[b]

            def p_chunk(idx):
                b, c = chunks[idx]
                emit_xt_dma(idx + 2)
                x_t = xtiles.pop(idx)
                qt, ktt, vt = get_qkv(b)
                c0 = c * 512
                HD = HPC * DH
                HKD = KT * DH
                for h in range(HPC):
                    for w_t, dst, bias_t in (
                        (wq_t, qt, bq_t),
                        (wk_t, ktt, bk_t),
                    ):
                        ps_t = ps_st.tile([128, 1024], F32, tag="st",
                                          name="qk_ps")
                        ps = ps_t[:, :512]
                        for k in range(KT):
                            nc.tensor.matmul(
                                ps,
                                w_t[:, h * HKD + k * DH : h * HKD
                                    + (k + 1) * DH],
                                x_t[:, k * 512 : (k + 1) * 512],
                                start=(k == 0),
                                stop=(k == KT - 1),
                            )
                        nc.vector.tensor_scalar_add(
                            dst[:, h, c0 : c0 + 512],
                            ps,
                            bias_t[:, h : h + 1],
                        )
                for s in range(4):
                    ps = ps_ms.tile([128, HD], F32, tag="ms",
                                    name="v_ps")
                    for k in range(KT):
                        nc.tensor.matmul(
                            ps[:],
                            x_t[:, k * 512 + s * 128 : k * 512
                                + (s + 1) * 128],
                            wv_t[:, k * HD : (k + 1) * HD],
                            start=(k == 0),
                            stop=(k == KT - 1),
                        )
                    nc.scalar.copy(vt[:, c * 4 + s, :], ps[:])

            def a_chunk(idx):
                b, qc = chunks[idx]
                qt, ktt, vt = get_qkv(b)
                n_kt = 4 * (qc + 1)
                at_sb = []
                for h in range(HPC):
                    hsl = slice(h * DH, (h + 1) * DH)
                    attn_ps = ps_at.tile([128, 512], F32, tag="at",
                                         name="attn_ps")
                    e_sb = ep.tile([128, 512], BF16, tag="E", name="e_sb")
                    for p in range(n_kt // 2):
                        i0, i1 = 2 * p, 2 * p + 1
                        lo0 = 128 * (i0 - 4 * qc) if i0 >= 4 * qc else 0
                        lo1 = 128 * (i1 - 4 * qc) if i1 >= 4 * qc else 0
                        st = ps_st.tile([128, 1024], F32, tag="st",
                                        name="st")
                        ex = expp.tile([128, 1024], BF16, tag="exp",
                                       name="ex")
                        for j, (i, lo) in enumerate(((i0, lo0), (i1, lo1))):
                            off = j * 512
                            nc.tensor.matmul(
                                st[:, off + lo : off + 512],
                                ktt[:, h, i * 128 : (i + 1) * 128],
                                qt[:, h, qc * 512 + lo : (qc + 1) * 512],
                                start=True,
                                stop=True,
                            )
                            if i >= 4 * qc:  # diagonal tile: add mask
                                nc.vector.tensor_tensor(
                                    st[:, off + lo : off + lo + 128],
                                    st[:, off + lo : off + lo + 128],
                                    mask_t[:],
                                    op=ALU.add,
                                )
                        # one exp over both halves (middle cols unused)
                        nc.scalar.activation(
                            ex[:, lo0:], st[:, lo0:], AF.Exp, scale=SCALE
                        )
                        # accumulate denominator tile E on DVE (bf16 2x)
                        if p == 0:
                            nc.vector.tensor_copy(
                                e_sb[:, lo0:], ex[:, lo0:512]
                            )
                        else:
                            nc.vector.tensor_tensor(
                                e_sb[:, lo0:],
                                e_sb[:, lo0:],
                                ex[:, lo0:512],
                                op=ALU.add,
                            )
                        nc.vector.tensor_tensor(
                            e_sb[:, lo1:],
                            e_sb[:, lo1:],
                            ex[:, 512 + lo1 :],
                            op=ALU.add,
                        )
                        nc.tensor.matmul(
                            attn_ps[:, lo0:],
                            vt[:, i0, hsl],
                            ex[:, lo0:512],
                            start=(p == 0),
                            stop=False,
                        )
                        nc.tensor.matmul(
                            attn_ps[:, lo1:],
                            vt[:, i1, hsl],
                            ex[:, 512 + lo1 :],
                            start=False,
                            stop=(p == n_kt // 2 - 1),
                        )
                    # rbden lives in the st pool (a half pair-tile):
                    # keeps the ms ring free for proj/outproj groups
                    rbden_t = ps_st.tile([128, 1024], F32, tag="st",
                                         name="rbden")
                    rbden = rbden_t[:, :512]
                    nc.tensor.matmul(
                        rbden, ones_m[:], e_sb[:], start=True, stop=True
                    )
                    rc_sb = attnp.tile([128, 512], F32, tag="rc",
                                       name="rc_sb")
                    nc.vector.reciprocal_approx_fast(
                        out=rc_sb[:], in_=rbden
                    )
                    a_sb = attnp.tile([128, 512], BF16, tag="attnT",
                                      name="a_sb")
                    nc.vector.tensor_tensor(
                        a_sb[:], attn_ps[:], rc_sb[:], op=ALU.mult
                    )
                    at_sb.append(a_sb)
                return at_sb

            def outproj(idx, at_sb):
                b, qc = chunks[idx]
                for qs in range(4):
                    row0 = qc * 512 + qs * 128
                    o_t = ostp.tile([128, D], BF16, tag="ost", name="o_t")
                    for nch in range(4):
                        ps = ps_ms.tile([128, 512], F32, tag="ms",
                                        name="op_ps")
                        for h in range(HPC):
                            nc.tensor.matmul(
                                ps[:],
                                at_sb[h][:, qs * 128 : (qs + 1) * 128],
                                wo_t[:, h * D + nch * 512 : h * D
                                     + (nch + 1) * 512],
                                start=(h == 0),
                                stop=(h == HPC - 1),
                            )
                        o0 = nch * 512
                        nc.vector.tensor_copy(
                            o_t[:, o0 : o0 + 256], ps[:, :256]
                        )
                        nc.scalar.copy(
                            o_t[:, o0 + 256 : o0 + 512], ps[:, 256:]
                        )
                    nc.gpsimd.dma_start(
                        out[b, row0 : row0 + 128, :], o_t[:]
                    )

            pending = None
            for idx in range(len(chunks)):
                p_chunk(idx)
                if pending is not None:
                    outproj(pending[0], pending[1])
                at = a_chunk(idx)
                pending = (idx, at)
            outproj(pending[0], pending[1])
    nc.compile()
    return nc


def _get_nc():
    if "nc" not in _NC_CACHE:
        _NC_CACHE["nc"] = _build()
    return _NC_CACHE["nc"]


def kernel(x, mask, Wq, bq, Wk, bk, Wv, bv, Wo, bo):
    import ml_dtypes

    bf16 = ml_dtypes.bfloat16
    x = np.asarray(x, dtype=np.float32)
    Wq = np.asarray(Wq, dtype=np.float32)
    Wk = np.asarray(Wk, dtype=np.float32)
    Wv = np.asarray(Wv, dtype=np.float32)
    Wo = np.asarray(Wo, dtype=np.float32)
    bq = np.asarray(bq, dtype=np.float32)
    bk = np.asarray(bk, dtype=np.float32)
    bv = np.asarray(bv, dtype=np.float32)
    bo = np.asarray(bo, dtype=np.float32)

    nc = _get_nc()

    # x -> [B, NQC, 128(model block), KT*512] with model on partitions:
    # element (b, c, p, k*512+s) = x[b, c*512+s, k*128+p]
    xt_np = (
        np.ascontiguousarray(
            x.reshape(B, NQC, 512, KT, 128).transpose(0, 1, 4, 3, 2)
        )
        .reshape(B, NQC, 128, KT * 512)
        .astype(bf16)
    )
    kl = np.arange(128)
    mblk = np.where(kl[:, None] <= kl[None, :], 0.0, NEG).astype(np.float32)
    onem = np.ones((128, 128), dtype=bf16)

    def pack_w(W, cols):
        # k-major: [128, k*(HPC*DH) + hd]; partition p holds row k*128+p
        return np.ascontiguousarray(
            W[:, cols].reshape(KT, 128, HPC * DH).transpose(1, 0, 2)
        ).reshape(128, KT * HPC * DH).astype(bf16)

    def pack_w_headmajor(W, cols):
        # head-major: [128, h*(KT*DH) + k*DH + d]
        w = W[:, cols].reshape(KT, 128, HPC, DH)  # (k, p, h, d)
        return np.ascontiguousarray(
            w.transpose(1, 2, 0, 3)  # (p, h, k, d)
        ).reshape(128, HPC * KT * DH).astype(bf16)

    in_maps = []
    for c in range(NCORES):
        cols = slice(c * HPC * DH, (c + 1) * HPC * DH)
        wo_c = np.ascontiguousarray(
            Wo[cols, :].reshape(HPC, 128, D).transpose(1, 0, 2)
        ).reshape(128, HPC * D).astype(bf16)
        in_maps.append(
            {
                "xt": xt_np,
                "wq": pack_w_headmajor(Wq, cols),
                "wk": pack_w_headmajor(Wk, cols),
                "wv": pack_w(Wv, cols),
                "wo": wo_c,
                "bq2": np.ascontiguousarray(
                    bq[cols].reshape(HPC, 128).T
                ),
                "bk2": np.ascontiguousarray(
                    bk[cols].reshape(HPC, 128).T
                ),
                "mblk": mblk,
                "onem": onem,
            }
        )

    trace = bool(int(os.environ.get("BASS_ATTN_TRACE", "0")))
    try:
        res = run_bass_kernel_spmd(
            nc, in_maps, core_ids=list(range(NCORES)), trace=trace
        )
    except Exception:
        # transient device errors (e.g. a wedged core from a prior run)
        # usually clear on retry
        res = run_bass_kernel_spmd(
            nc, in_maps, core_ids=list(range(NCORES)), trace=trace
        )
    if trace:
        _NC_CACHE["last_result"] = res

    acc = res.results[0]["out"].astype(np.float64)
    for c in range(1, NCORES):
        acc += res.results[c]["out"].astype(np.float64)
    # bv's effect: softmax rows sum to 1, so attn = attn_nobv + bv per head
    # -> out += bv @ Wo (exact). bo added directly.
    corr = (bv.astype(np.float64) @ Wo.astype(np.float64)) + bo.astype(
        np.float64
    )
    acc += corr
    return acc.astype(np.float32)


# revision 30
# speedup vs baseline: 1.0259x; 1.0016x over previous
"""Causal self-attention on 8 trn2 NeuronCores.

Sharding: tensor-parallel over heads (2 heads per core, both batches).
Each core computes Q/K/V projections for its heads (column-parallel),
causal attention, and a row-parallel partial of the output projection.
Host sums the 8 partials and adds the bias terms (bo, and bv@Wo which
is exact because softmax rows sum to 1).

v2 changes vs the 439us baseline:
  - all matmuls bf16 (same PE rate as fp32r, FWL weight loads, half DMA)
  - consolidated DMAs: one or two descriptors per weight tensor, one
    16KB/partition descriptor per (batch, chunk) of x; host pre-packs
    layouts so no small-element gather DMAs remain
  - dedicated DMA queues: sync=x chunks, gpsimd=weights+output
  - PE warmup matmul stream at t=0 so HAM reaches K=8/8 before real work
  - paired score tiles: one [128,1024] PSUM (2 banks) per two key tiles,
    one exp instruction covering both (halves ACT per-instruction cost)
  - softmax denominator: exp tiles accumulated on DVE into E, a single
    ones-matmul per (head, chunk) instead of one per key tile
  - output staged [128,2048] bf16 in SBUF, one DMA per row block

Layout choices (partition dim first):
  xt   : x transposed -> (model 128-blocks on partitions, seq free)
  Qt/Kt: (head_dim on partitions, seq free)        [proj lhsT = W block]
  V    : (seq keys on partitions, head_dim free)   [proj lhsT = xt block]
  scores: St = (keys, queries) tiles = Kt_blk.T @ Qt_chunk
  exp(St) feeds P@V directly:  attnT = V_blk.T @ exp  (head_dim, queries)
  denominator: ones(128,128).T @ E where E = sum_i exp_i (DVE)
  out-proj: lhsT = attnT block, rhs = Wo rows slice -> (queries, model)
"""

import os

import numpy as np

import concourse.bass as bass
import concourse.mybir as mybir
import concourse.tile as tile
from concourse import bacc
from concourse.bass_utils import run_bass_kernel_spmd

F32 = mybir.dt.float32
BF16 = mybir.dt.bfloat16
AF = mybir.ActivationFunctionType
ALU = mybir.AluOpType

B = 2
S = 2048
D = 2048
H = 16
DH = 128
NCORES = 8
HPC = H // NCORES  # heads per core = 2
KT = D // 128  # 16 contraction tiles for projections
NQC = S // 512  # 4 query chunks per sequence
SCALE = 1.0 / np.sqrt(DH)
NEG = -1e9

_NC_CACHE = {}


def _build():
    nc = bacc.Bacc(None, target_bir_lowering=False, debug=False)

    # host pre-packed inputs (see kernel() below)
    xt = nc.dram_tensor("xt", [B, NQC, 128, KT * 512], BF16,
                        kind="ExternalInput")
    wq = nc.dram_tensor("wq", [128, KT * HPC * DH], BF16,
                        kind="ExternalInput")
    wk = nc.dram_tensor("wk", [128, KT * HPC * DH], BF16,
                        kind="ExternalInput")
    wv = nc.dram_tensor("wv", [128, KT * HPC * DH], BF16,
                        kind="ExternalInput")
    wo = nc.dram_tensor("wo", [128, HPC * D], BF16, kind="ExternalInput")
    bq2 = nc.dram_tensor("bq2", [128, HPC], F32, kind="ExternalInput")
    bk2 = nc.dram_tensor("bk2", [128, HPC], F32, kind="ExternalInput")
    mblk = nc.dram_tensor("mblk", [128, 128], F32, kind="ExternalInput")
    onem = nc.dram_tensor("onem", [128, 128], BF16, kind="ExternalInput")
    out = nc.dram_tensor("out", [B, S, D], BF16, kind="ExternalOutput")

    with tile.TileContext(nc) as tc:
        with (
            tc.tile_pool(name="const", bufs=1) as constp,
            tc.tile_pool(name="xtp", bufs=3) as xtp,
            tc.tile_pool(name="qkv", bufs=2) as qkvp,
            tc.tile_pool(name="expp", bufs=6) as expp,
            tc.tile_pool(name="ep", bufs=2) as ep,
            tc.tile_pool(name="attnp", bufs=5) as attnp,
            tc.tile_pool(name="ostp", bufs=3) as ostp,
            # PSUM bank budget (8): st-pairs 2x2 + attn 2 + misc 2
            tc.tile_pool(name="ps_st", bufs=2, space="PSUM") as ps_st,
            tc.tile_pool(name="ps_at", bufs=2, space="PSUM") as ps_at,
            tc.tile_pool(name="ps_ms", bufs=2, space="PSUM") as ps_ms,
        ):
            # ---- constants: critical first-chunk weights first ----
            bq_t = constp.tile([128, HPC], F32, tag="bq")
            bk_t = constp.tile([128, HPC], F32, tag="bk")
            mask_t = constp.tile([128, 128], F32, tag="mask")
            ones_m = constp.tile([128, 128], BF16, tag="ones_m")

            # wq/wk packed per-head: [128, h*(KT*DH) + k*DH + d] so the
            # first half-DMA delivers all of head 0 (consumption order).
            HKD = KT * DH  # 2048 = one head's weight cols
            HD = HPC * DH
            wq_t = constp.tile([128, HPC * HKD], BF16, tag="wq")
            wk_t = constp.tile([128, HPC * HKD], BF16, tag="wk")
            wv_t = constp.tile([128, KT * HD], BF16, tag="wv")
            wo_t = constp.tile([128, HPC * D], BF16, tag="wo")
            nc.gpsimd.dma_start(wq_t[:, :HKD], wq[:, :HKD])
            nc.gpsimd.dma_start(wk_t[:, :HKD], wk[:, :HKD])
            nc.gpsimd.dma_start(bq_t[:], bq2[:])
            nc.gpsimd.dma_start(bk_t[:], bk2[:])
            nc.gpsimd.dma_start(mask_t[:], mblk[:])
            nc.gpsimd.dma_start(ones_m[:], onem[:])
            nc.gpsimd.dma_start(wq_t[:, HKD:], wq[:, HKD:])
            nc.gpsimd.dma_start(wk_t[:, HKD:], wk[:, HKD:])
            hw = KT * HD // 2
            nc.gpsimd.dma_start(wv_t[:, :hw], wv[:, :hw])
            nc.gpsimd.dma_start(wv_t[:, hw:], wv[:, hw:])
            nc.gpsimd.dma_start(
                wo_t[:, : HPC * D // 2], wo[:, : HPC * D // 2]
            )
            nc.gpsimd.dma_start(
                wo_t[:, HPC * D // 2 :], wo[:, HPC * D // 2 :]
            )

            # ---- warmup: ACT exp table + ~5us of PE activity ----
            warm_sb = constp.tile([128, 512], BF16, tag="warm_sb")
            nc.vector.memset(warm_sb[:], 0)
            warm_t = constp.tile([128, 1], F32, tag="warm")
            nc.scalar.activation(warm_t[:], warm_sb[:, 0:1], AF.Exp,
                                 scale=0.0)
            warm_ps = ps_ms.tile([128, 512], F32, tag="ms", name="warm_ps")
            NWARM = 14
            for i in range(NWARM):
                nc.tensor.matmul(
                    warm_ps[:],
                    warm_sb[:, :128],
                    warm_sb[:],
                    start=(i == 0),
                    stop=(i == NWARM - 1),
                )

            # ---- chunk sequence ----
            chunks = [(b, c) for b in range(B) for c in range(NQC)]
            xtiles = {}

            def emit_xt_dma(idx):
                if idx >= len(chunks):
                    return
                b, c = chunks[idx]
                x_t = xtp.tile([128, KT * 512], BF16, tag="xt",
                               name=f"xt{b}_{c}")
                qtr = KT * 512 // 4
                for q in range(4):
                    nc.sync.dma_start(
                        x_t[:, q * qtr : (q + 1) * qtr],
                        xt[b, c, :, q * qtr : (q + 1) * qtr],
                    )
                xtiles[idx] = x_t

            emit_xt_dma(0)
            emit_xt_dma(1)

            # per-batch persistent tiles (double-buffered across batches)
            qkts = {}

            def get_qkv(b):
                if b not in qkts:
                    qt = qkvp.tile([128, HPC, S], BF16, tag="qt",
                                   name=f"qt{b}")
                    ktt = qkvp.tile([128, HPC, S], BF16, tag="ktt",
                                    name=f"ktt{b}")
                    vt = qkvp.tile([128, S // 128, HPC * DH], BF16, tag="vt",
                                   name=f"vt{b}")
                    qkts[b] = (qt, ktt, vt)
                return qkts[b]

            def p_chunk(idx):
                b, c = chunks[idx]
                emit_xt_dma(idx + 2)
                x_t = xtiles.pop(idx)
                qt, ktt, vt = get_qkv(b)
                c0 = c * 512
                HD = HPC * DH
                HKD = KT * DH
                for h in range(HPC):
                    for w_t, dst, bias_t in (
                        (wq_t, qt, bq_t),
                        (wk_t, ktt, bk_t),
                    ):
                        ps_t = ps_st.tile([128, 1024], F32, tag="st",
                                          name="qk_ps")
                        ps = ps_t[:, :512]
                        for k in range(KT):
                            nc.tensor.matmul(
                                ps,
                                w_t[:, h * HKD + k * DH : h * HKD
                                    + (k + 1) * DH],
                                x_t[:, k * 512 : (k + 1) * 512],
                                start=(k == 0),
                                stop=(k == KT - 1),
                            )
                        nc.vector.tensor_scalar_add(
                            dst[:, h, c0 : c0 + 512],
                            ps,
                            bias_t[:, h : h + 1],
                        )
                for s in range(4):
                    ps = ps_ms.tile([128, HD], F32, tag="ms",
                                    name="v_ps")
                    for k in range(KT):
                        nc.tensor.matmul(
                            ps[:],
                            x_t[:, k * 512 + s * 128 : k * 512
                                + (s + 1) * 128],
                            wv_t[:, k * HD : (k + 1) * HD],
                            start=(k == 0),
                            stop=(k == KT - 1),
                        )
                    nc.scalar.copy(vt[:, c * 4 + s, :], ps[:])

            def a_chunk(idx):
                b, qc = chunks[idx]
                qt, ktt, vt = get_qkv(b)
                n_kt = 4 * (qc + 1)
                at_sb = []
                for h in range(HPC):
                    hsl = slice(h * DH, (h + 1) * DH)
                    attn_ps = ps_at.tile([128, 512], F32, tag="at",
                                         name="attn_ps")
                    e_sb = ep.tile([128, 1024], BF16, tag="E",
                                   name="e_sb")
                    for p in range(n_kt // 2):
                        i0, i1 = 2 * p, 2 * p + 1
                        lo0 = 128 * (i0 - 4 * qc) if i0 >= 4 * qc else 0
                        lo1 = 128 * (i1 - 4 * qc) if i1 >= 4 * qc else 0
                        st = ps_st.tile([128, 1024], F32, tag="st",
                                        name="st")
                        ex = expp.tile([128, 1024], BF16, tag="exp",
                                       name="ex")
                        for j, (i, lo) in enumerate(((i0, lo0), (i1, lo1))):
                            off = j * 512
                            nc.tensor.matmul(
                                st[:, off + lo : off + 512],
                                ktt[:, h, i * 128 : (i + 1) * 128],
                                qt[:, h, qc * 512 + lo : (qc + 1) * 512],
                                start=True,
                                stop=True,
                            )
                            if i >= 4 * qc:  # diagonal tile: add mask
                                nc.vector.tensor_tensor(
                                    st[:, off + lo : off + lo + 128],
                                    st[:, off + lo : off + lo + 128],
                                    mask_t[:],
                                    op=ALU.add,
                                )
                        # one exp over both halves (middle cols unused)
                        nc.scalar.activation(
                            ex[:, lo0:], st[:, lo0:], AF.Exp, scale=SCALE
                        )
                        # accumulate denominators on DVE into a
                        # [128,1024] double tile: one op per full pair
                        if i1 < 4 * qc:  # full pair, lo0 == lo1 == 0
                            if p == 0:
                                nc.vector.tensor_copy(e_sb[:], ex[:])
                            else:
                                nc.vector.tensor_tensor(
                                    e_sb[:], e_sb[:], ex[:], op=ALU.add
                                )
                        elif p == 0:  # qc == 0: first pair is diagonal
                            nc.vector.tensor_copy(
                                e_sb[:, lo0:512], ex[:, lo0:512]
                            )
                            nc.vector.memset(e_sb[:, 512 : 512 + lo1], 0)
                            nc.vector.tensor_copy(
                                e_sb[:, 512 + lo1 :], ex[:, 512 + lo1 :]
                            )
                        else:  # diagonal pair: sliced adds
                            nc.vector.tensor_tensor(
                                e_sb[:, lo0:512],
                                e_sb[:, lo0:512],
                                ex[:, lo0:512],
                                op=ALU.add,
                            )
                            nc.vector.tensor_tensor(
                                e_sb[:, 512 + lo1 :],
                                e_sb[:, 512 + lo1 :],
                                ex[:, 512 + lo1 :],
                                op=ALU.add,
                            )
                        nc.tensor.matmul(
                            attn_ps[:, lo0:],
                            vt[:, i0, hsl],
                            ex[:, lo0:512],
                            start=(p == 0),
                            stop=False,
                        )
                        nc.tensor.matmul(
                            attn_ps[:, lo1:],
                            vt[:, i1, hsl],
                            ex[:, 512 + lo1 :],
                            start=False,
                            stop=(p == n_kt // 2 - 1),
                        )
                    # rbden lives in the st pool (a half pair-tile):
                    # keeps the ms ring free for proj/outproj groups
                    rbden_t = ps_st.tile([128, 1024], F32, tag="st",
                                         name="rbden")
                    rbden = rbden_t[:, :512]
                    nc.tensor.matmul(
                        rbden, ones_m[:], e_sb[:, :512],
                        start=True, stop=False,
                    )
                    nc.tensor.matmul(
                        rbden, ones_m[:], e_sb[:, 512:],
                        start=False, stop=True,
                    )
                    rc_sb = attnp.tile([128, 512], F32, tag="rc",
                                       name="rc_sb")
                    nc.vector.reciprocal_approx_fast(
                        out=rc_sb[:], in_=rbden
                    )
                    a_sb = attnp.tile([128, 512], BF16, tag="attnT",
                                      name="a_sb")
                    nc.vector.tensor_tensor(
                        a_sb[:], attn_ps[:], rc_sb[:], op=ALU.mult
                    )
                    at_sb.append(a_sb)
                return at_sb

            def outproj(idx, at_sb):
                b, qc = chunks[idx]
                for qs in range(4):
                    row0 = qc * 512 + qs * 128
                    o_t = ostp.tile([128, D], BF16, tag="ost", name="o_t")
                    for nch in range(4):
                        ps = ps_ms.tile([128, 512], F32, tag="ms",
                                        name="op_ps")
                        for h in range(HPC):
                            nc.tensor.matmul(
                                ps[:],
                                at_sb[h][:, qs * 128 : (qs + 1) * 128],
                                wo_t[:, h * D + nch * 512 : h * D
                                     + (nch + 1) * 512],
                                start=(h == 0),
                                stop=(h == HPC - 1),
                            )
                        osl = o_t[:, nch * 512 : (nch + 1) * 512]
                        if nch % 2 == 0:
                            nc.vector.tensor_copy(osl, ps[:])
                        else:
                            nc.scalar.copy(osl, ps[:])
                    nc.gpsimd.dma_start(
                        out[b, row0 : row0 + 128, :], o_t[:]
                    )

            pending = None
            for idx in range(len(chunks)):
                p_chunk(idx)
                if pending is not None:
                    outproj(pending[0], pending[1])
                at = a_chunk(idx)
                pending = (idx, at)
            outproj(pending[0], pending[1])
    nc.compile()
    return nc


def _get_nc():
    if "nc" not in _NC_CACHE:
        _NC_CACHE["nc"] = _build()
    return _NC_CACHE["nc"]


def kernel(x, mask, Wq, bq, Wk, bk, Wv, bv, Wo, bo):
    import ml_dtypes

    bf16 = ml_dtypes.bfloat16
    x = np.asarray(x, dtype=np.float32)
    Wq = np.asarray(Wq, dtype=np.float32)
    Wk = np.asarray(Wk, dtype=np.float32)
    Wv = np.asarray(Wv, dtype=np.float32)
    Wo = np.asarray(Wo, dtype=np.float32)
    bq = np.asarray(bq, dtype=np.float32)
    bk = np.asarray(bk, dtype=np.float32)
    bv = np.asarray(bv, dtype=np.float32)
    bo = np.asarray(bo, dtype=np.float32)

    nc = _get_nc()

    # x -> [B, NQC, 128(model block), KT*512] with model on partitions:
    # element (b, c, p, k*512+s) = x[b, c*512+s, k*128+p]
    xt_np = (
        np.ascontiguousarray(
            x.reshape(B, NQC, 512, KT, 128).transpose(0, 1, 4, 3, 2)
        )
        .reshape(B, NQC, 128, KT * 512)
        .astype(bf16)
    )
    kl = np.arange(128)
    mblk = np.where(kl[:, None] <= kl[None, :], 0.0, NEG).astype(np.float32)
    onem = np.ones((128, 128), dtype=bf16)

    def pack_w(W, cols):
        # k-major: [128, k*(HPC*DH) + hd]; partition p holds row k*128+p
        return np.ascontiguousarray(
            W[:, cols].reshape(KT, 128, HPC * DH).transpose(1, 0, 2)
        ).reshape(128, KT * HPC * DH).astype(bf16)

    def pack_w_headmajor(W, cols):
        # head-major: [128, h*(KT*DH) + k*DH + d]
        w = W[:, cols].reshape(KT, 128, HPC, DH)  # (k, p, h, d)
        return np.ascontiguousarray(
            w.transpose(1, 2, 0, 3)  # (p, h, k, d)
        ).reshape(128, HPC * KT * DH).astype(bf16)

    in_maps = []
    for c in range(NCORES):
        cols = slice(c * HPC * DH, (c + 1) * HPC * DH)
        wo_c = np.ascontiguousarray(
            Wo[cols, :].reshape(HPC, 128, D).transpose(1, 0, 2)
        ).reshape(128, HPC * D).astype(bf16)
        in_maps.append(
            {
                "xt": xt_np,
                "wq": pack_w_headmajor(Wq, cols),
                "wk": pack_w_headmajor(Wk, cols),
                "wv": pack_w(Wv, cols),
                "wo": wo_c,
                "bq2": np.ascontiguousarray(
                    bq[cols].reshape(HPC, 128).T
                ),
                "bk2": np.ascontiguousarray(
                    bk[cols].reshape(HPC, 128).T
                ),
                "mblk": mblk,
                "onem": onem,
            }
        )

    trace = bool(int(os.environ.get("BASS_ATTN_TRACE", "0")))
    try:
        res = run_bass_kernel_spmd(
            nc, in_maps, core_ids=list(range(NCORES)), trace=trace
        )
    except Exception:
        # transient device errors (e.g. a wedged core from a prior run)
        # usually clear on retry
        res = run_bass_kernel_spmd(
            nc, in_maps, core_ids=list(range(NCORES)), trace=trace
        )
    if trace:
        _NC_CACHE["last_result"] = res

    acc = res.results[0]["out"].astype(np.float64)
    for c in range(1, NCORES):
        acc += res.results[c]["out"].astype(np.float64)
    # bv's effect: softmax rows sum to 1, so attn = attn_nobv + bv per head
    # -> out += bv @ Wo (exact). bo added directly.
    corr = (bv.astype(np.float64) @ Wo.astype(np.float64)) + bo.astype(
        np.float64
    )
    acc += corr
    return acc.astype(np.float32)
